# revision 23
# baseline (speedup 1.0000x reference)
"""Trainium2 Bass kernel for ChamferPccRateDistortionLoss.

Problem (hardcoded shapes):
  x_hat       [8, 4096, 3]  f32
  pos         [8, 4096, 3]  f32
  likelihoods [8, 192, 1024] f32
Returns (loss, bpp_loss, rec_loss) scalars, matching:
  bpp  = sum(-log2(lik)) / (8*4096)
  rec  = mean_b( mean_p min_q d2 + mean_q min_p d2 ),  d2 = clamped sq-dists
  loss = rec + bpp

Sharding: data-parallel over batch. Core b computes its batch's partial
results (per-point row mins, a col-min map, and the ln-likelihood sum);
the tiny final reduction happens on the host after gathering.

Per-core dataflow:
  - d2[p,q] computed on TensorE as one K=16 bf16 matmul using hi/lo
    splitting of fp32 (error ~2^-17, exact enough for fp32-level rel-err),
    with the -2xy / |x|^2 / |y|^2 terms folded in as extra K rows.
    4-way tile_position row packing (K=16 fits in one 32-row group).
  - ScalarE (ACT) drains PSUM -> SBUF fp16 (it is otherwise idle).
  - VectorE does BOTH min passes as 2x-mode tensor_tensor running-min
    updates (row: per-slab R accumulator; col: global C accumulator),
    with 4x-mode tensor_copy for first-touch initialization.
  - GpSimd (Pool) folds each slab's R accumulator 512->1 (DVE stays on
    the big passes).
"""

import math
from contextlib import ExitStack

import numpy as np

import concourse.bacc as bacc
import concourse.bass as bass
import concourse.mybir as mybir
import concourse.tile as tile
from concourse.bass_utils import run_bass_kernel_spmd
from concourse.masks import make_identity

B, P, D = 8, 4096, 3
LC, LL = 192, 1024            # likelihoods per batch
NCORES = 8
PCH = 128                     # points per chunk (partition dim)
NCHUNK = P // PCH             # 32 chunks
NSLAB = NCHUNK // 4           # 8 slabs of 4 chunks (one per row-group)
QWIN = 512                    # q points per matmul (one PSUM bank)
NQW = P // QWIN               # 8 q windows
K = 16                        # contraction rows used

f32 = mybir.dt.float32
f16 = mybir.dt.float16
bf16 = mybir.dt.bfloat16

_CACHED = None


def _build_program():
    """Build the per-core Bass program (same program on all 8 cores)."""
    nc = bacc.Bacc(
        "TRN2",
        target_bir_lowering=False,
        debug=False,
        enable_asserts=True,
        num_devices=NCORES,
    )

    x_dram = nc.dram_tensor("x_in", [P, D], f32, kind="ExternalInput").ap()
    y_dram = nc.dram_tensor("y_in", [P, D], f32, kind="ExternalInput").ap()
    lik_dram = nc.dram_tensor("lik_in", [LC, LL], f32, kind="ExternalInput").ap()

    out_c = nc.dram_tensor("out_c", [128, P], f16, kind="ExternalOutput").ap()
    out_r = nc.dram_tensor("out_r", [128, NCHUNK], f32, kind="ExternalOutput").ap()
    out_l = nc.dram_tensor("out_l", [128, 12], f32, kind="ExternalOutput").ap()

    with tile.TileContext(nc) as tc, ExitStack() as ctx:
        const_pool = ctx.enter_context(tc.tile_pool(name="const", bufs=1))
        in_pool = ctx.enter_context(tc.tile_pool(name="ins", bufs=1))
        feat_pool = ctx.enter_context(tc.tile_pool(name="feat", bufs=1))
        acc_pool = ctx.enter_context(tc.tile_pool(name="acc", bufs=1))
        r_pool = ctx.enter_context(tc.tile_pool(name="rrow", bufs=2))
        s_pool = ctx.enter_context(tc.tile_pool(name="sbig", bufs=4))
        psum_pool = ctx.enter_context(tc.tile_pool(name="psum", bufs=2, space="PSUM"))
        dram_pool = ctx.enter_context(tc.tile_pool(name="dscr", bufs=1, space="DRAM"))

        # ---- constants ----
        ident = const_pool.tile([128, 128], bf16)
        make_identity(nc, ident[:])
        # REPALL[:, r2*128:(r2+1)*128] is the selector REP_r2 with
        # REP_r2[c, 32g+k] = 1 iff c == 32*r2+k : as a matmul stationary it
        # broadcasts FyT's row-group r2 to all four 32-row groups, replacing
        # the DRAM-bounce replication of FyR with PE work.
        REPALL = const_pool.tile([128, 512], bf16)
        nc.vector.memset(REPALL[:], 0.0)
        for r2 in range(4):
            for g in range(4):
                nc.vector.tensor_copy(
                    REPALL[32 * r2:32 * r2 + 32,
                           r2 * 128 + 32 * g:r2 * 128 + 32 * g + 32],
                    ident[32 * r2:32 * r2 + 32, 32 * r2:32 * r2 + 32],
                )

        # ---- inputs ----
        xm = in_pool.tile([128, NCHUNK * D], f32)   # x point-major: [l, (c d)]
        ym = in_pool.tile([128, NCHUNK * D], f32)
        lik = in_pool.tile([128, LC * LL // 128], f32)
        # y side first: it has the longest critical path to the first matmul
        # (features -> transposes -> DRAM-bounce replication)
        nc.sync.dma_start(
            ym[:].rearrange("l (c d) -> l c d", d=D),
            y_dram.rearrange("(c l) d -> l c d", l=128),
        )
        nc.sync.dma_start(
            xm[:].rearrange("l (c d) -> l c d", d=D),
            x_dram.rearrange("(c l) d -> l c d", l=128),
        )
        nc.gpsimd.dma_start(
            lik[:], lik_dram.rearrange("a b -> (a b)").rearrange("(p f) -> p f", p=128)
        )

        # ---- features (point-major), hi/lo split ----
        # K-row pairing (lhsT row k  x  rhs row k):
        #   0-2 : -2*xh_d  * yh_d      3-5 : -2*xh_d * yl_d
        #   6-8 : -2*xl_d  * yh_d      9-11: -2*xl_d * yl_d
        #   12  : x2h * 1              13  : x2l * 1
        #   14  : 1 * y2h              15  : 1 * y2l
        Fx = feat_pool.tile([128, NCHUNK * 32], bf16)   # [l, (c k)] k-stride 32
        Fy = feat_pool.tile([128, NCHUNK * 32], bf16)
        nc.vector.memset(Fy[:], 1.0)   # rows 12..15 defaults (1s); 16..31 unused
        nc.vector.memset(Fx[:], 1.0)
        Fxv = Fx[:].rearrange("p (c k) -> p c k", k=32)
        Fyv = Fy[:].rearrange("p (c k) -> p c k", k=32)
        mul = mybir.AluOpType.mult
        sub = mybir.AluOpType.subtract

        def build_features(m, Fv, scale, hi_sl, lo_sl, n_sl, name, dup_eng=None):
            """Write hi/lo split features (scaled) + norm hi/lo directly into
            the strided F slots. hi = bf16(scale*m); lo = scale*m - hi.
            Copy-type ops can go on `dup_eng` (e.g. idle ScalarE)."""
            if dup_eng is nc.scalar:
                dup_copy = nc.scalar.copy
            else:
                dup_copy = nc.vector.tensor_copy
            mv = m[:].rearrange("p (c d) -> p c d", d=D)
            sq = feat_pool.tile([128, NCHUNK * D], f32, tag=f"sq{name}")
            n2 = feat_pool.tile([128, NCHUNK], f32, tag=f"n2{name}")
            nc.vector.tensor_tensor(sq[:], m[:], m[:], op=mul)
            nc.vector.tensor_reduce(
                n2[:], sq[:].rearrange("p (c d) -> p c d", d=D),
                axis=mybir.AxisListType.X, op=mybir.AluOpType.add,
            )
            n2v = n2[:].rearrange("p (c o) -> p c o", o=1)
            hi0 = Fv[:, :, hi_sl[0]:hi_sl[0] + D]
            nc.vector.tensor_scalar_mul(hi0, mv, scale)
            dup_copy(Fv[:, :, hi_sl[1]:hi_sl[1] + D], hi0)
            lo0 = Fv[:, :, lo_sl[0]:lo_sl[0] + D]
            nc.vector.scalar_tensor_tensor(lo0, mv, scale, hi0, op0=mul, op1=sub)
            dup_copy(Fv[:, :, lo_sl[1]:lo_sl[1] + D], lo0)
            n2h = Fv[:, :, n_sl:n_sl + 1]
            dup_copy(n2h, n2v)
            nc.vector.scalar_tensor_tensor(
                Fv[:, :, n_sl + 1:n_sl + 2], n2v, 1.0, n2h, op0=mul, op1=sub)

        # ---- transpose features to [K, point] layout ----
        # slab s of Fx columns [s*128,(s+1)*128) = chunks 4s..4s+3, 32 feats each
        # transpose -> psum[32*(c%4)+k, l]; chunk 4s+r lands at partitions 32r.
        # FyT is replicated to every 32-row group (so any row-group can stream
        # any q) via a DRAM bounce. Emission order keeps the y-chain (longest
        # path to first matmul) dense; x-side work overlaps the bounce.
        FxT = feat_pool.tile([128, NSLAB * 128], bf16)
        FyR = feat_pool.tile([128, P], bf16)
        FyT = feat_pool.tile([128, NSLAB * 128], bf16)

        # y side: yh at 0:3 & 6:9, yl at 3:6 & 9:12 ; y2 hi/lo at 14,15
        build_features(ym, Fyv, 1.0, (0, 6), (3, 9), 14, "y", dup_eng=nc.scalar)
        # bpp: sum of ln(likelihood), chunked into 12 small ACT ops that fill
        # early-prologue ScalarE idle time without ever blocking the
        # drain/prologue chain. Host sums the 12 partial accumulators.
        ln_scr = in_pool.tile([128, LC * LL // 128], f16)
        ln_acc = acc_pool.tile([128, 12], f32)
        for j in range(12):
            nc.scalar.activation(
                ln_scr[:, j * 128:(j + 1) * 128],
                lik[:, j * 128:(j + 1) * 128],
                mybir.ActivationFunctionType.Ln,
                accum_out=ln_acc[:, j:j + 1],
            )
        # x side: -2xh at 0:3 & 3:6 ; -2xl at 6:9 & 9:12 ; x2 hi/lo at 12,13
        build_features(xm, Fxv, -2.0, (0, 3), (6, 9), 12, "x")
        # Transposes: all 8 slabs of each side go into one wide PSUM tile and
        # leave with a single wide copy (FyT on ScalarE, feeding the REP
        # matmuls; FxT on VectorE, idle here) — no per-slab PSUM round-trip.
        # All transposes precede the REP matmuls in the PE queue so window
        # matmuls can start as soon as FyR's first block is replicated.
        ptTy = psum_pool.tile([128, NSLAB * 128], bf16, tag="pt")
        for s in range(NSLAB):
            nc.tensor.transpose(
                ptTy[:, s * 128:(s + 1) * 128],
                Fy[:, s * 128:(s + 1) * 128], ident[:])
        nc.scalar.copy(FyT[:], ptTy[:])
        ptTx = psum_pool.tile([128, NSLAB * 128], bf16, tag="pt")
        for s in range(NSLAB):
            nc.tensor.transpose(
                ptTx[:, s * 128:(s + 1) * 128],
                Fx[:, s * 128:(s + 1) * 128], ident[:])
        nc.vector.tensor_copy(FxT[:], ptTx[:])
        # replicate each 32-row group of FyT to all groups with REP selector
        # matmuls on the PE — no DRAM bounce, no DMA-completion stalls.
        for r2 in range(4):
            ptR = psum_pool.tile([128, 1024], f32, tag="pt")
            for s in range(NSLAB):
                nc.tensor.matmul(
                    ptR[:, s * 128:(s + 1) * 128],
                    lhsT=REPALL[:, r2 * 128:(r2 + 1) * 128],
                    rhs=FyT[:, s * 128:(s + 1) * 128],
                    start=True,
                    stop=True,
                )
            dst = FyR[:, r2 * 1024:(r2 + 1) * 1024]
            if r2 < 2:
                nc.scalar.copy(dst, ptR[:])
            else:
                nc.vector.tensor_copy(dst, ptR[:])

        # ---- accumulators ----
        # C16 [p, c, qperm]: col-min map per chunk-within-slab, accumulated
        # over slabs with 2x tensor_tensor min (first slab = 4x copy).
        C16 = acc_pool.tile([128, 4 * P], f16)
        C16v = C16[:].rearrange("p (c q) -> p c q", c=4)
        RS = acc_pool.tile([128, NCHUNK], f32)  # folded row mins per (chunk)

        mn = mybir.AluOpType.min

        # ---- main loop ----
        # Drains go straight into accumulator tiles where possible:
        #   slab 0          : drain -> C16 slice (col init for free)
        #   slabs 1-7, w=0  : drain -> R        (row init for free)
        # Windows 1-7 of slabs 1-7 drain pairwise into one S2 tile so each
        # col update covers two windows in a single 2x op. Slab 7 folds C16
        # 4->1 per finalized q-slice and streams out_c immediately.
        def finalize_c16(lo, hi):
            """Fold C16 chunks 4->1 over q-slice [lo,hi) and DMA it out."""
            nc.vector.tensor_tensor(
                C16v[:, 0, lo:hi], C16v[:, 0, lo:hi], C16v[:, 1, lo:hi], op=mn)
            nc.vector.tensor_tensor(
                C16v[:, 2, lo:hi], C16v[:, 2, lo:hi], C16v[:, 3, lo:hi], op=mn)
            nc.vector.tensor_tensor(
                C16v[:, 0, lo:hi], C16v[:, 0, lo:hi], C16v[:, 2, lo:hi], op=mn)
            nc.sync.dma_start(out_c[:, lo:hi], C16v[:, 0, lo:hi])

        def mm_window(s, w, dst_ap):
            """Matmul the 4 chunks of window w into PSUM, drain to dst_ap."""
            pt = psum_pool.tile([128, 4 * QWIN], f32, tag="pt")
            for r in range(4):
                nc.tensor.matmul(
                    pt[:, r * QWIN:(r + 1) * QWIN],
                    lhsT=FxT[32 * r:32 * r + K, s * 128:(s + 1) * 128],
                    rhs=FyR[32 * r:32 * r + K, w * QWIN:(w + 1) * QWIN],
                    start=True,
                    stop=True,
                    tile_position=(32 * r, 0),
                )
            nc.scalar.copy(dst_ap, pt[:].rearrange("p (c q) -> p c q", c=4))

        for s in range(NSLAB):
            last = s == NSLAB - 1
            # per-slab row-min accumulator [p, c, qmod512]
            R = r_pool.tile([128, 4 * QWIN], f16, tag="R")
            Rv = R[:].rearrange("p (c q) -> p c q", c=4)
            if s == 0:
                for w in range(NQW):
                    Cw = C16v[:, :, w * QWIN:(w + 1) * QWIN]
                    mm_window(s, w, Cw)
                    if w == 0:
                        nc.vector.tensor_copy(Rv, Cw)
                    else:
                        nc.vector.tensor_tensor(Rv, Rv, Cw, op=mn)
            else:
                # w = 0: drain into R, fold into C16
                mm_window(s, 0, Rv)
                nc.vector.tensor_tensor(
                    C16v[:, :, 0:QWIN], C16v[:, :, 0:QWIN], Rv, op=mn)
                if last:
                    finalize_c16(0, QWIN)
                # w = 1..6 in pairs; w = 7 single
                for w0 in (1, 3, 5, 7):
                    npair = 1 if w0 == 7 else 2
                    S2 = s_pool.tile([128, 2 * 4 * QWIN], f16)
                    S2v = S2[:].rearrange("p (c q) -> p c q", c=4)
                    for i in range(npair):
                        w = w0 + i
                        half = S2v[:, :, i * QWIN:(i + 1) * QWIN]
                        mm_window(s, w, half)
                        nc.vector.tensor_tensor(Rv, Rv, half, op=mn)
                    lo, hi = w0 * QWIN, (w0 + npair) * QWIN
                    nc.vector.tensor_tensor(
                        C16v[:, :, lo:hi], C16v[:, :, lo:hi],
                        S2v[:, :, 0:npair * QWIN], op=mn)
                    if last:
                        finalize_c16(lo, hi)
            # fold R 512 -> 32 per chunk (2x tensor_tensor halving), then a
            # tiny 1x reduce finishes 32 -> 1. (Pool can't run TensorTensor.)
            for width in (256, 128, 64, 32):
                nc.vector.tensor_tensor(
                    Rv[:, :, 0:width], Rv[:, :, 0:width],
                    Rv[:, :, width:2 * width], op=mn)
            nc.vector.tensor_reduce(
                RS[:, 4 * s:4 * s + 4], Rv[:, :, 0:32],
                axis=mybir.AxisListType.X, op=mn,
            )

        nc.sync.dma_start(out_r, RS[:])
        nc.sync.dma_start(out_l, ln_acc[:])

    nc.compile()
    return nc


def _get_program():
    global _CACHED
    if _CACHED is None:
        _CACHED = _build_program()
    return _CACHED


def run_on_cores(x_hat, pos, likelihoods, **spmd_kwargs):
    """Compile (cached) + run on 8 cores; returns BassKernelResults."""
    nc = _get_program()
    in_maps = [
        {
            "x_in": np.ascontiguousarray(x_hat[b], dtype=np.float32),
            "y_in": np.ascontiguousarray(pos[b], dtype=np.float32),
            "lik_in": np.ascontiguousarray(likelihoods[b], dtype=np.float32),
        }
        for b in range(B)
    ]
    return run_bass_kernel_spmd(nc, in_maps, core_ids=list(range(NCORES)), **spmd_kwargs)


def combine(results):
    """Host-side reduction of per-core partials -> (loss, bpp, rec)."""
    cham = []
    ln_sum = 0.0
    for r in results:
        rowmins = np.maximum(r["out_r"].astype(np.float64), 0.0)  # [128, 32]
        colmins = np.maximum(
            r["out_c"].astype(np.float64).min(axis=0), 0.0
        )  # [4096]
        cham.append(rowmins.mean() + colmins.mean())
        ln_sum += float(r["out_l"].astype(np.float64).sum())
    rec = float(np.mean(cham))
    bpp = -ln_sum / math.log(2.0) / (B * P)
    loss = rec + bpp
    return (
        np.float32(loss),
        np.float32(bpp),
        np.float32(rec),
    )


def kernel(x_hat, pos, likelihoods):
    res = run_on_cores(x_hat, pos, likelihoods)
    return combine(res.results)


# revision 25
# speedup vs baseline: 1.0216x; 1.0216x over previous
"""Trainium2 Bass kernel for ChamferPccRateDistortionLoss.

Problem (hardcoded shapes):
  x_hat       [8, 4096, 3]  f32
  pos         [8, 4096, 3]  f32
  likelihoods [8, 192, 1024] f32
Returns (loss, bpp_loss, rec_loss) scalars, matching:
  bpp  = sum(-log2(lik)) / (8*4096)
  rec  = mean_b( mean_p min_q d2 + mean_q min_p d2 ),  d2 = clamped sq-dists
  loss = rec + bpp

Sharding: data-parallel over batch. Core b computes its batch's partial
results (per-point row mins, a col-min map, and the ln-likelihood sum);
the tiny final reduction happens on the host after gathering.

Per-core dataflow:
  - d2[p,q] computed on TensorE as one K=16 bf16 matmul using hi/lo
    splitting of fp32 (error ~2^-17, exact enough for fp32-level rel-err),
    with the -2xy / |x|^2 / |y|^2 terms folded in as extra K rows.
    4-way tile_position row packing (K=16 fits in one 32-row group).
  - ScalarE (ACT) drains PSUM -> SBUF fp16 (it is otherwise idle).
  - VectorE does BOTH min passes as 2x-mode tensor_tensor running-min
    updates (row: per-slab R accumulator; col: global C accumulator),
    with 4x-mode tensor_copy for first-touch initialization.
  - GpSimd (Pool) folds each slab's R accumulator 512->1 (DVE stays on
    the big passes).
"""

import math
from contextlib import ExitStack

import numpy as np

import concourse.bacc as bacc
import concourse.bass as bass
import concourse.mybir as mybir
import concourse.tile as tile
from concourse.bass_utils import run_bass_kernel_spmd
from concourse.masks import make_identity

B, P, D = 8, 4096, 3
LC, LL = 192, 1024            # likelihoods per batch
NCORES = 8
PCH = 128                     # points per chunk (partition dim)
NCHUNK = P // PCH             # 32 chunks
NSLAB = NCHUNK // 4           # 8 slabs of 4 chunks (one per row-group)
QWIN = 512                    # q points per matmul (one PSUM bank)
NQW = P // QWIN               # 8 q windows
K = 16                        # contraction rows used

f32 = mybir.dt.float32
f16 = mybir.dt.float16
bf16 = mybir.dt.bfloat16

_CACHED = None


def _build_program():
    """Build the per-core Bass program (same program on all 8 cores)."""
    nc = bacc.Bacc(
        "TRN2",
        target_bir_lowering=False,
        debug=False,
        enable_asserts=True,
        num_devices=NCORES,
    )

    x_dram = nc.dram_tensor("x_in", [P, D], f32, kind="ExternalInput").ap()
    y_dram = nc.dram_tensor("y_in", [P, D], f32, kind="ExternalInput").ap()
    lik_dram = nc.dram_tensor("lik_in", [LC, LL], f32, kind="ExternalInput").ap()

    out_c = nc.dram_tensor("out_c", [128, P], f16, kind="ExternalOutput").ap()
    out_r = nc.dram_tensor("out_r", [128, NCHUNK], f32, kind="ExternalOutput").ap()
    out_l = nc.dram_tensor("out_l", [128, 12], f32, kind="ExternalOutput").ap()

    with tile.TileContext(nc) as tc, ExitStack() as ctx:
        const_pool = ctx.enter_context(tc.tile_pool(name="const", bufs=1))
        in_pool = ctx.enter_context(tc.tile_pool(name="ins", bufs=1))
        feat_pool = ctx.enter_context(tc.tile_pool(name="feat", bufs=1))
        acc_pool = ctx.enter_context(tc.tile_pool(name="acc", bufs=1))
        r_pool = ctx.enter_context(tc.tile_pool(name="rrow", bufs=2))
        s_pool = ctx.enter_context(tc.tile_pool(name="sbig", bufs=4))
        psum_pool = ctx.enter_context(tc.tile_pool(name="psum", bufs=2, space="PSUM"))
        dram_pool = ctx.enter_context(tc.tile_pool(name="dscr", bufs=1, space="DRAM"))

        # ---- constants ----
        ident = const_pool.tile([128, 128], bf16)
        make_identity(nc, ident[:])
        # REPALL[:, r2*128:(r2+1)*128] is the selector REP_r2 with
        # REP_r2[c, 32g+k] = 1 iff c == 32*r2+k : as a matmul stationary it
        # broadcasts FyT's row-group r2 to all four 32-row groups, replacing
        # the DRAM-bounce replication of FyR with PE work.
        REPALL = const_pool.tile([128, 512], bf16)
        nc.vector.memset(REPALL[:], 0.0)
        for r2 in range(4):
            for g in range(4):
                nc.vector.tensor_copy(
                    REPALL[32 * r2:32 * r2 + 32,
                           r2 * 128 + 32 * g:r2 * 128 + 32 * g + 32],
                    ident[32 * r2:32 * r2 + 32, 32 * r2:32 * r2 + 32],
                )

        # ---- inputs ----
        xm = in_pool.tile([128, NCHUNK * D], f32)   # x point-major: [l, (c d)]
        ym = in_pool.tile([128, NCHUNK * D], f32)
        lik = in_pool.tile([128, LC * LL // 128], f32)
        # y side first: it has the longest critical path to the first matmul
        # (features -> transposes -> DRAM-bounce replication)
        nc.sync.dma_start(
            ym[:].rearrange("l (c d) -> l c d", d=D),
            y_dram.rearrange("(c l) d -> l c d", l=128),
        )
        nc.sync.dma_start(
            xm[:].rearrange("l (c d) -> l c d", d=D),
            x_dram.rearrange("(c l) d -> l c d", l=128),
        )
        nc.gpsimd.dma_start(
            lik[:], lik_dram.rearrange("a b -> (a b)").rearrange("(p f) -> p f", p=128)
        )

        # ---- features (point-major), hi/lo split ----
        # K-row pairing (lhsT row k  x  rhs row k):
        #   0-2 : -2*xh_d  * yh_d      3-5 : -2*xh_d * yl_d
        #   6-8 : -2*xl_d  * yh_d      9-11: -2*xl_d * yl_d
        #   12  : x2h * 1              13  : x2l * 1
        #   14  : 1 * y2h              15  : 1 * y2l
        Fx = feat_pool.tile([128, NCHUNK * 32], bf16)   # [l, (c k)] k-stride 32
        Fy = feat_pool.tile([128, NCHUNK * 32], bf16)
        nc.vector.memset(Fy[:], 1.0)   # rows 12..15 defaults (1s); 16..31 unused
        nc.vector.memset(Fx[:], 1.0)
        Fxv = Fx[:].rearrange("p (c k) -> p c k", k=32)
        Fyv = Fy[:].rearrange("p (c k) -> p c k", k=32)
        mul = mybir.AluOpType.mult
        sub = mybir.AluOpType.subtract

        def build_features(m, Fv, scale, hi_sl, lo_sl, n_sl, name, dup_eng=None):
            """Write hi/lo split features (scaled) + norm hi/lo directly into
            the strided F slots. hi = bf16(scale*m); lo = scale*m - hi.
            Copy-type ops can go on `dup_eng` (e.g. idle ScalarE)."""
            if dup_eng is nc.scalar:
                dup_copy = nc.scalar.copy
            else:
                dup_copy = nc.vector.tensor_copy
            mv = m[:].rearrange("p (c d) -> p c d", d=D)
            sq = feat_pool.tile([128, NCHUNK * D], f32, tag=f"sq{name}")
            n2 = feat_pool.tile([128, NCHUNK], f32, tag=f"n2{name}")
            nc.vector.tensor_tensor(sq[:], m[:], m[:], op=mul)
            nc.vector.tensor_reduce(
                n2[:], sq[:].rearrange("p (c d) -> p c d", d=D),
                axis=mybir.AxisListType.X, op=mybir.AluOpType.add,
            )
            n2v = n2[:].rearrange("p (c o) -> p c o", o=1)
            hi0 = Fv[:, :, hi_sl[0]:hi_sl[0] + D]
            nc.vector.tensor_scalar_mul(hi0, mv, scale)
            dup_copy(Fv[:, :, hi_sl[1]:hi_sl[1] + D], hi0)
            lo0 = Fv[:, :, lo_sl[0]:lo_sl[0] + D]
            nc.vector.scalar_tensor_tensor(lo0, mv, scale, hi0, op0=mul, op1=sub)
            dup_copy(Fv[:, :, lo_sl[1]:lo_sl[1] + D], lo0)
            n2h = Fv[:, :, n_sl:n_sl + 1]
            dup_copy(n2h, n2v)
            nc.vector.scalar_tensor_tensor(
                Fv[:, :, n_sl + 1:n_sl + 2], n2v, 1.0, n2h, op0=mul, op1=sub)

        # ---- transpose features to [K, point] layout ----
        # slab s of Fx columns [s*128,(s+1)*128) = chunks 4s..4s+3, 32 feats each
        # transpose -> psum[32*(c%4)+k, l]; chunk 4s+r lands at partitions 32r.
        # FyT is replicated to every 32-row group (so any row-group can stream
        # any q) via a DRAM bounce. Emission order keeps the y-chain (longest
        # path to first matmul) dense; x-side work overlaps the bounce.
        FxT = feat_pool.tile([128, NSLAB * 128], bf16)
        FyR = feat_pool.tile([128, P], bf16)
        FyT = feat_pool.tile([128, NSLAB * 128], bf16)

        # y side: yh at 0:3 & 6:9, yl at 3:6 & 9:12 ; y2 hi/lo at 14,15
        build_features(ym, Fyv, 1.0, (0, 6), (3, 9), 14, "y", dup_eng=nc.scalar)
        # x side: -2xh at 0:3 & 3:6 ; -2xl at 6:9 & 9:12 ; x2 hi/lo at 12,13
        build_features(xm, Fxv, -2.0, (0, 3), (6, 9), 12, "x")
        # Transposes: all 8 slabs of each side go into one wide PSUM tile and
        # leave with a single wide copy (FyT on ScalarE, feeding the REP
        # matmuls; FxT on VectorE, idle here) — no per-slab PSUM round-trip.
        # All transposes precede the REP matmuls in the PE queue so window
        # matmuls can start as soon as FyR's first block is replicated.
        ptTy = psum_pool.tile([128, NSLAB * 128], bf16, tag="pt")
        for s in range(NSLAB):
            nc.tensor.transpose(
                ptTy[:, s * 128:(s + 1) * 128],
                Fy[:, s * 128:(s + 1) * 128], ident[:])
        nc.scalar.copy(FyT[:], ptTy[:])
        ptTx = psum_pool.tile([128, NSLAB * 128], bf16, tag="pt")
        for s in range(NSLAB):
            nc.tensor.transpose(
                ptTx[:, s * 128:(s + 1) * 128],
                Fx[:, s * 128:(s + 1) * 128], ident[:])
        nc.vector.tensor_copy(FxT[:], ptTx[:])
        # replicate each 32-row group of FyT to all groups with REP selector
        # matmuls on the PE — no DRAM bounce, no DMA-completion stalls.
        for r2 in range(4):
            ptR = psum_pool.tile([128, 1024], f32, tag="pt")
            for s in range(NSLAB):
                nc.tensor.matmul(
                    ptR[:, s * 128:(s + 1) * 128],
                    lhsT=REPALL[:, r2 * 128:(r2 + 1) * 128],
                    rhs=FyT[:, s * 128:(s + 1) * 128],
                    start=True,
                    stop=True,
                )
            dst = FyR[:, r2 * 1024:(r2 + 1) * 1024]
            if r2 < 2:
                nc.scalar.copy(dst, ptR[:])
            else:
                nc.vector.tensor_copy(dst, ptR[:])

        # ---- accumulators ----
        # C16 [p, c, qperm]: col-min map per chunk-within-slab, accumulated
        # over slabs with 2x tensor_tensor min (first slab = 4x copy).
        C16 = acc_pool.tile([128, 4 * P], f16)
        C16v = C16[:].rearrange("p (c q) -> p c q", c=4)
        RS = acc_pool.tile([128, NCHUNK], f32)  # folded row mins per (chunk)

        mn = mybir.AluOpType.min

        # ---- main loop ----
        # Drains go straight into accumulator tiles where possible:
        #   slab 0          : drain -> C16 slice (col init for free)
        #   slabs 1-7, w=0  : drain -> R        (row init for free)
        # Windows 1-7 of slabs 1-7 drain pairwise into one S2 tile so each
        # col update covers two windows in a single 2x op. Slab 7 folds C16
        # 4->1 per finalized q-slice and streams out_c immediately.
        def finalize_c16(lo, hi):
            """Fold C16 chunks 4->1 over q-slice [lo,hi) and DMA it out."""
            nc.vector.tensor_tensor(
                C16v[:, 0, lo:hi], C16v[:, 0, lo:hi], C16v[:, 1, lo:hi], op=mn)
            nc.vector.tensor_tensor(
                C16v[:, 2, lo:hi], C16v[:, 2, lo:hi], C16v[:, 3, lo:hi], op=mn)
            nc.vector.tensor_tensor(
                C16v[:, 0, lo:hi], C16v[:, 0, lo:hi], C16v[:, 2, lo:hi], op=mn)
            nc.sync.dma_start(out_c[:, lo:hi], C16v[:, 0, lo:hi])

        def mm_window(s, w, dst_ap):
            """Matmul the 4 chunks of window w into PSUM, drain to dst_ap."""
            pt = psum_pool.tile([128, 4 * QWIN], f32, tag="pt")
            for r in range(4):
                nc.tensor.matmul(
                    pt[:, r * QWIN:(r + 1) * QWIN],
                    lhsT=FxT[32 * r:32 * r + K, s * 128:(s + 1) * 128],
                    rhs=FyR[32 * r:32 * r + K, w * QWIN:(w + 1) * QWIN],
                    start=True,
                    stop=True,
                    tile_position=(32 * r, 0),
                )
            nc.scalar.copy(dst_ap, pt[:].rearrange("p (c q) -> p c q", c=4))

        for s in range(NSLAB):
            last = s == NSLAB - 1
            # per-slab row-min accumulator [p, c, qmod512]
            R = r_pool.tile([128, 4 * QWIN], f16, tag="R")
            Rv = R[:].rearrange("p (c q) -> p c q", c=4)
            if s == 0:
                for w in range(NQW):
                    Cw = C16v[:, :, w * QWIN:(w + 1) * QWIN]
                    mm_window(s, w, Cw)
                    if w == 0:
                        nc.vector.tensor_copy(Rv, Cw)
                    else:
                        nc.vector.tensor_tensor(Rv, Rv, Cw, op=mn)
            else:
                # w = 0: drain into R, fold into C16
                mm_window(s, 0, Rv)
                nc.vector.tensor_tensor(
                    C16v[:, :, 0:QWIN], C16v[:, :, 0:QWIN], Rv, op=mn)
                if last:
                    finalize_c16(0, QWIN)
                # w = 1..6 in pairs; w = 7 single
                for w0 in (1, 3, 5, 7):
                    npair = 1 if w0 == 7 else 2
                    S2 = s_pool.tile([128, 2 * 4 * QWIN], f16)
                    S2v = S2[:].rearrange("p (c q) -> p c q", c=4)
                    for i in range(npair):
                        w = w0 + i
                        half = S2v[:, :, i * QWIN:(i + 1) * QWIN]
                        mm_window(s, w, half)
                        nc.vector.tensor_tensor(Rv, Rv, half, op=mn)
                    lo, hi = w0 * QWIN, (w0 + npair) * QWIN
                    nc.vector.tensor_tensor(
                        C16v[:, :, lo:hi], C16v[:, :, lo:hi],
                        S2v[:, :, 0:npair * QWIN], op=mn)
                    if last:
                        finalize_c16(lo, hi)
            # fold R 512 -> 32 per chunk (2x tensor_tensor halving), then a
            # tiny 1x reduce finishes 32 -> 1. (Pool can't run TensorTensor.)
            for width in (256, 128, 64, 32):
                nc.vector.tensor_tensor(
                    Rv[:, :, 0:width], Rv[:, :, 0:width],
                    Rv[:, :, width:2 * width], op=mn)
            nc.vector.tensor_reduce(
                RS[:, 4 * s:4 * s + 4], Rv[:, :, 0:32],
                axis=mybir.AxisListType.X, op=mn,
            )

        nc.sync.dma_start(out_r, RS[:])

        # ---- bpp: sum of ln(likelihood) per partition, chunked into 12
        # small ACT ops so the scheduler fills ScalarE bubbles with them
        # instead of ever blocking the drain/prologue chain. Host sums the
        # 12 partial accumulators.
        ln_scr = in_pool.tile([128, LC * LL // 128], f16)
        ln_acc = acc_pool.tile([128, 12], f32)
        for j in range(12):
            nc.scalar.activation(
                ln_scr[:, j * 128:(j + 1) * 128],
                lik[:, j * 128:(j + 1) * 128],
                mybir.ActivationFunctionType.Ln,
                accum_out=ln_acc[:, j:j + 1],
            )
        nc.sync.dma_start(out_l, ln_acc[:])

    nc.compile()
    return nc


def _get_program():
    global _CACHED
    if _CACHED is None:
        _CACHED = _build_program()
    return _CACHED


def run_on_cores(x_hat, pos, likelihoods, **spmd_kwargs):
    """Compile (cached) + run on 8 cores; returns BassKernelResults."""
    nc = _get_program()
    in_maps = [
        {
            "x_in": np.ascontiguousarray(x_hat[b], dtype=np.float32),
            "y_in": np.ascontiguousarray(pos[b], dtype=np.float32),
            "lik_in": np.ascontiguousarray(likelihoods[b], dtype=np.float32),
        }
        for b in range(B)
    ]
    return run_bass_kernel_spmd(nc, in_maps, core_ids=list(range(NCORES)), **spmd_kwargs)


def combine(results):
    """Host-side reduction of per-core partials -> (loss, bpp, rec)."""
    cham = []
    ln_sum = 0.0
    for r in results:
        rowmins = np.maximum(r["out_r"].astype(np.float64), 0.0)  # [128, 32]
        colmins = np.maximum(
            r["out_c"].astype(np.float64).min(axis=0), 0.0
        )  # [4096]
        cham.append(rowmins.mean() + colmins.mean())
        ln_sum += float(r["out_l"].astype(np.float64).sum())
    rec = float(np.mean(cham))
    bpp = -ln_sum / math.log(2.0) / (B * P)
    loss = rec + bpp
    return (
        np.float32(loss),
        np.float32(bpp),
        np.float32(rec),
    )


def kernel(x_hat, pos, likelihoods):
    res = run_on_cores(x_hat, pos, likelihoods)
    return combine(res.results)
